# revision 37
# baseline (speedup 1.0000x reference)
"""NTXEnt (intra-sample) loss kernel for Trainium2, 8-core data-parallel.

Math (matches the jax reference):
  inp [C=8, V=2, B=4096, D=512] fp32
  xn = inp / max(||inp||_D, 1e-12)
  sim[i,b,jv] = <xn[i,0,b], xn[jv,b]> / T          (T = 0.1)
  loss[i,b]   = log( sum over jv of exp(sim) with the (j==i,v=0) self
                     column zeroed; the (i,1) pos column counts once ) - pos
  answer = mean over (i, b).

Sharding: pure data parallel over B (4096 -> 8 cores x 512).  Each core
computes per-(b,i) partial losses [128, 4*8]; the host sums in float64
and divides by C*B.

Engine split (v3).  The 92 pair-dots + 16 self-dots (norms) per 128-row
chunk are organized in anchor groups.  Per group of n pairs:
  1. DVE bf16 tensor_tensor product (2x mode): P[128, n, 512]
  2. TensorE reduces 512 -> SLICE via identity-stationary matmuls that
     accumulate D-slices into PSUM:  PS[b, k, s] = sum_j P[b, k, s + j*SLICE]
  3. final SLICE -> 1: either one DVE tensor_reduce per group (1x, but
     only n*SLICE elements) or ScalarE Copy+accum per pair, balancing
     both engines.
This beats the 1x-only scalar_tensor_tensor dot (756ns/pair measured) at
~270 (DVE) + ~220 (PE, idle otherwise) + ~100-480 (evac) ns/pair.
rsqrt via ln/exp with 1/T=10 folded in: r' = exp(-0.5*ln(nn) + 0.5*ln10).
A single ACT table set (natural_log_exp_and_others: ln/exp/copy) is
pre-loaded so walrus inserts no per-chunk table switches.
"""

import math
import os
import numpy as np

C, V, B, D = 8, 2, 4096, 512
NCORES = 8
B_LOC = B // NCORES            # 512
P = 128                        # partitions per chunk

SLICE = int(os.environ.get("NTX_SLICE", "64"))   # PE first-stage output width
N_SC_PAIRS = int(os.environ.get("NTX_SC_PAIRS", "42"))  # pairs evac'd by ScalarE
N_SC_LAST = int(os.environ.get("NTX_SC_LAST", "12"))   # ... on the last chunk


def _build_kernel(b_loc=B_LOC, slice_=SLICE, n_sc_pairs=N_SC_PAIRS,
                  n_sc_last=N_SC_LAST):
    from contextlib import ExitStack

    import concourse.bacc as bacc
    import concourse.tile as tile
    import concourse.mybir as mybir
    from concourse.hw_specs import get_activation_tables

    f32 = mybir.dt.float32
    bf16 = mybir.dt.bfloat16
    Alu = mybir.AluOpType
    Act = mybir.ActivationFunctionType

    nchunk = b_loc // P
    nsl = D // slice_
    half_ln10 = 0.5 * math.log(10.0)

    nc = bacc.Bacc("TRN2", target_bir_lowering=False, debug=False)
    x_d = nc.dram_tensor("inp", [C, V, b_loc, D], f32, kind="ExternalInput")
    id_d = nc.dram_tensor("ident", [P, P], f32, kind="ExternalInput")
    o_d = nc.dram_tensor("out", [P, nchunk * C], f32, kind="ExternalOutput")

    # Pre-load the one ACT table set that contains ln+exp+copy, so the
    # compile-time fixpoint never needs to insert per-chunk switches.
    tset = list(get_activation_tables(nc.m.arch).keys())
    set_id = tset.index("natural_log_exp_and_others")
    nc.scalar.add_instruction(mybir.InstLoadActFuncSet(
        name=nc.get_next_instruction_name(), act_func_set_id=set_id,
        ins=[], outs=[]))

    # groups: (iv_a, [iv_c...], kind, anchor_i) — kind in {v1, tri},
    # interleaved so ScalarE-evac'd and DVE-evac'd pairs alternate.
    # Norms run separately on ScalarE (Square+accum straight off X).
    groups = []
    v1g = [(2 * i, [2 * j + 1 for j in range(C)], "v1", i) for i in range(C)]
    trig = [(2 * i, [2 * j for j in range(i + 1, C)], "tri", i)
            for i in range(C - 1)]
    for k in range(C):
        groups.append(v1g[k])
        if k < len(trig):
            groups.append(trig[k])
    # chunk-0 variant: its X arrives as two half-DMAs (crops 0-3, 4-7);
    # lead with tri sub-groups confined to crops 0-3 so DVE starts ~3us
    # before the second half lands.
    groups0 = [(0, [2, 4, 6], "tri", 0), (2, [4, 6], "tri", 1),
               (4, [6], "tri", 2)]
    for k in range(C):
        groups0.append(v1g[k])
        if k < 3:
            groups0.append((2 * k, [8, 10, 12, 14], "tri", k))
        elif k < len(trig):
            groups0.append(trig[k])

    with tile.TileContext(nc) as tc, ExitStack() as ctx:
        xp = ctx.enter_context(tc.tile_pool(name="x", bufs=2))
        pp = ctx.enter_context(tc.tile_pool(name="prod", bufs=5))
        psp = ctx.enter_context(tc.psum_pool(name="ps", bufs=7))
        scr_s = ctx.enter_context(tc.tile_pool(name="scr_s", bufs=3))
        small = ctx.enter_context(tc.tile_pool(name="small", bufs=2))
        outp = ctx.enter_context(tc.tile_pool(name="outp", bufs=1))

        loss_out = outp.tile([P, nchunk * C], f32)
        bias_t = outp.tile([P, 1], f32)
        nc.vector.memset(bias_t[:, :], half_ln10)
        ident = outp.tile([P, P], bf16)
        nc.gpsimd.dma_start(out=ident[:, :], in_=id_d.ap())  # casts f32->bf16

        x_ap = x_d.ap()

        def load_chunk(c):
            # two half-loads so chunk-0 compute can start ~6us earlier
            X = xp.tile([P, C * V, D], bf16, tag="X")
            src = x_ap[:, :, c * P:(c + 1) * P, :].rearrange(
                "i v b d -> b (i v) d")
            nc.gpsimd.dma_start(out=X[:, 0:8, :], in_=src[:, 0:8, :])
            nc.gpsimd.dma_start(out=X[:, 8:16, :], in_=src[:, 8:16, :])
            return X

        def emit_groups(X, sc_budget, glist):
            G = small.tile([P, C, C * V], f32)
            Gf = G[:, :, :].rearrange("p a b -> p (a b)")   # [128, 128] view

            # ---- norms^2 on ScalarE straight off bf16 X, then the r-chain
            nn = small.tile([P, C * V], f32)
            for iv in range(C * V):
                sq = scr_s.tile([P, D], bf16, tag="sq")
                nc.scalar.activation(out=sq[:, :], in_=X[:, iv, :],
                                     func=Act.Square,
                                     accum_out=nn[:, iv:iv + 1])
            lnn = small.tile([P, C * V], f32)
            nc.scalar.activation(out=lnn[:, :], in_=nn[:, :], func=Act.Ln)
            r = small.tile([P, C * V], f32)
            nc.scalar.activation(out=r[:, :], in_=lnn[:, :], func=Act.Exp,
                                 scale=-0.5, bias=bias_t[:, :])

            sc_quota = [0.0]    # Bresenham spread of the ScalarE evac budget
            for (iva, ivcs, kind, ai) in glist:
                n = len(ivcs)
                # ---- 1. products P[128, n, 512] bf16 (DVE TT, 2x)
                Pg = pp.tile([P, 8, D], bf16, tag="pg")
                in0 = X[:, iva, :].unsqueeze(1).broadcast_to([P, n, D])
                st = ivcs[1] - ivcs[0] if n > 1 else 1
                in1 = X[:, ivcs[0]:ivcs[-1] + 1:st, :]
                nc.vector.tensor_tensor(out=Pg[:, :n, :], in0=in0, in1=in1,
                                        op=Alu.mult)

                # ---- 2. PE: 512 -> slice_ accumulate-copy into PSUM
                PS = psp.tile([P, n, slice_], f32)
                for j in range(nsl):
                    nc.tensor.matmul(
                        out=PS[:, :, :], lhsT=ident[:, :],
                        rhs=Pg[:, :n, j * slice_:(j + 1) * slice_],
                        start=(j == 0), stop=(j == nsl - 1))

                # ---- 3. evac slice_ -> 1 into G.  The first sc_k pairs of
                # each group go to ScalarE (Copy+accum), the rest ride one
                # DVE tensor_reduce — pair-granular engine balance.
                j0, j1 = ivcs[0] // 2, ivcs[-1] // 2
                off = 1 if kind == "v1" else 0
                dst = Gf[:, 16 * ai + 2 * j0 + off:
                         16 * ai + 2 * j1 + off + 1:2]
                sc_quota[0] += n * (sc_budget / 92.0)
                sc_k = min(n, int(sc_quota[0]))
                sc_quota[0] -= sc_k
                for k in range(sc_k):
                    so = scr_s.tile([P, slice_], bf16, tag="so")
                    nc.scalar.activation(
                        out=so[:, :], in_=PS[:, k, :], func=Act.Copy,
                        accum_out=dst[:, k:k + 1])
                if sc_k < n:
                    nc.vector.tensor_reduce(
                        out=dst[:, sc_k:], in_=PS[:, sc_k:, :],
                        axis=mybir.AxisListType.X, op=Alu.add)

            return G, Gf, r

        def emit_tail(c, G, Gf, r):
            # ---- mirror v0 upper triangle -> lower: for offset k:
            #      src cols 18i+2k (i=0..8-k), dst cols 18i+16k
            for k in range(1, C):
                nk = C - k
                src_v = Gf[:, 2 * k: 2 * k + 18 * (nk - 1) + 1:18]
                dst_v = Gf[:, 16 * k: 16 * k + 18 * (nk - 1) + 1:18]
                nc.vector.tensor_copy(out=dst_v, in_=src_v)
            # ---- zero the (i,i,0) self columns (cols 18i)
            nc.vector.memset(Gf[:, 0:127:18], 0.0)

            # ---- RR[b, i, jv] = r'[b, 2i] * r'[b, jv]  (carries 1/T=10)
            RR = small.tile([P, C, C * V], f32)
            r_a = r[:, 0:C * V:2].unsqueeze(2).broadcast_to([P, C, C * V])
            r_c = r[:, :].unsqueeze(1).broadcast_to([P, C, C * V])
            nc.vector.tensor_tensor(out=RR[:, :, :], in0=r_a, in1=r_c,
                                    op=Alu.mult)

            # ---- sims = G * RR;  pos = sims[:, 18i+1]
            sims = small.tile([P, C, C * V], f32)
            nc.vector.tensor_tensor(out=sims[:, :, :], in0=G[:, :, :],
                                    in1=RR[:, :, :], op=Alu.mult)
            simsf = sims[:, :, :].rearrange("p a b -> p (a b)")
            pos = small.tile([P, C], f32)
            nc.vector.tensor_copy(out=pos[:, :], in_=simsf[:, 1:128:18])

            # ---- E = exp(sims); zero self cols; row-sum; loss = ln(D)-pos
            E = small.tile([P, C, C * V], f32)
            nc.scalar.activation(out=E[:, :, :], in_=sims[:, :, :],
                                 func=Act.Exp)
            Ef = E[:, :, :].rearrange("p a b -> p (a b)")
            nc.vector.memset(Ef[:, 0:127:18], 0.0)
            Dsum = small.tile([P, C], f32)
            nc.vector.tensor_reduce(out=Dsum[:, :], in_=E[:, :, :],
                                    axis=mybir.AxisListType.X, op=Alu.add)
            lnD = small.tile([P, C], f32)
            nc.scalar.activation(out=lnD[:, :], in_=Dsum[:, :], func=Act.Ln)
            nc.vector.tensor_tensor(
                out=loss_out[:, c * C:(c + 1) * C], in0=lnD[:, :],
                in1=pos[:, :], op=Alu.subtract)

        # software-pipeline: emit chunk c+1's product groups before chunk
        # c's (cheap but dependency-laden) tail, so the in-order DVE queue
        # never idles waiting for ScalarE's last evacuations.
        Xc = load_chunk(0)
        pending = None
        for c in range(nchunk):
            X = Xc
            if c + 1 < nchunk:
                Xc = load_chunk(c + 1)
            state = emit_groups(
                X, n_sc_last if c == nchunk - 1 else n_sc_pairs,
                groups0 if c == 0 else groups)
            if pending is not None:
                emit_tail(c - 1, *pending)
            pending = state
        emit_tail(nchunk - 1, *pending)

        nc.sync.dma_start(out=o_d.ap(), in_=loss_out[:, :])

    nc.compile()
    return nc


_CACHE = {}


def _get_nc(b_loc=B_LOC, slice_=SLICE, n_sc_pairs=N_SC_PAIRS,
            n_sc_last=N_SC_LAST):
    key = (b_loc, slice_, n_sc_pairs, n_sc_last)
    if key not in _CACHE:
        _CACHE[key] = _build_kernel(b_loc, slice_, n_sc_pairs, n_sc_last)
    return _CACHE[key]


def _run(inp, trace=False):
    from concourse.bass_utils import run_bass_kernel_spmd

    nc = _get_nc()
    ident = np.eye(P, dtype=np.float32)
    in_maps = []
    for k in range(NCORES):
        shard = np.ascontiguousarray(inp[:, :, k * B_LOC:(k + 1) * B_LOC, :],
                                     dtype=np.float32)
        in_maps.append({"inp": shard, "ident": ident})
    res = run_bass_kernel_spmd(nc, in_maps, list(range(NCORES)), trace=trace)
    total = np.float64(0.0)
    for m in res.results:
        total += m["out"].astype(np.float64).sum()
    loss = np.float32(total / (C * B))
    return loss, res


def kernel(inp):
    loss, _ = _run(np.asarray(inp), trace=False)
    return loss


# revision 39
# speedup vs baseline: 1.0731x; 1.0731x over previous
"""NTXEnt (intra-sample) loss kernel for Trainium2, 8-core data-parallel.

Math (matches the jax reference):
  inp [C=8, V=2, B=4096, D=512] fp32
  xn = inp / max(||inp||_D, 1e-12)
  sim[i,b,jv] = <xn[i,0,b], xn[jv,b]> / T          (T = 0.1)
  loss[i,b]   = log( sum over jv of exp(sim) with the (j==i,v=0) self
                     column zeroed; the (i,1) pos column counts once ) - pos
  answer = mean over (i, b).

Sharding: pure data parallel over B (4096 -> 8 cores x 512).  Each core
computes per-(b,i) partial losses [128, 4*8]; the host sums in float64
and divides by C*B.

Engine split (v3).  The 92 pair-dots + 16 self-dots (norms) per 128-row
chunk are organized in anchor groups.  Per group of n pairs:
  1. DVE bf16 tensor_tensor product (2x mode): P[128, n, 512]
  2. TensorE reduces 512 -> SLICE via identity-stationary matmuls that
     accumulate D-slices into PSUM:  PS[b, k, s] = sum_j P[b, k, s + j*SLICE]
  3. final SLICE -> 1: either one DVE tensor_reduce per group (1x, but
     only n*SLICE elements) or ScalarE Copy+accum per pair, balancing
     both engines.
This beats the 1x-only scalar_tensor_tensor dot (756ns/pair measured) at
~270 (DVE) + ~220 (PE, idle otherwise) + ~100-480 (evac) ns/pair.
rsqrt via ln/exp with 1/T=10 folded in: r' = exp(-0.5*ln(nn) + 0.5*ln10).
A single ACT table set (natural_log_exp_and_others: ln/exp/copy) is
pre-loaded so walrus inserts no per-chunk table switches.
"""

import math
import os
import numpy as np

C, V, B, D = 8, 2, 4096, 512
NCORES = 8
B_LOC = B // NCORES            # 512
P = 128                        # partitions per chunk

SLICE = int(os.environ.get("NTX_SLICE", "64"))   # PE first-stage output width
N_SC_PAIRS = int(os.environ.get("NTX_SC_PAIRS", "31"))  # pairs evac'd by ScalarE
N_SC_LAST = int(os.environ.get("NTX_SC_LAST", "12"))   # ... on the last chunk


def _build_kernel(b_loc=B_LOC, slice_=SLICE, n_sc_pairs=N_SC_PAIRS,
                  n_sc_last=N_SC_LAST):
    from contextlib import ExitStack

    import concourse.bacc as bacc
    import concourse.tile as tile
    import concourse.mybir as mybir
    from concourse.hw_specs import get_activation_tables

    f32 = mybir.dt.float32
    bf16 = mybir.dt.bfloat16
    Alu = mybir.AluOpType
    Act = mybir.ActivationFunctionType

    nchunk = b_loc // P
    nsl = D // slice_
    half_ln10 = 0.5 * math.log(10.0)

    nc = bacc.Bacc("TRN2", target_bir_lowering=False, debug=False)
    x_d = nc.dram_tensor("inp", [C, V, b_loc, D], f32, kind="ExternalInput")
    id_d = nc.dram_tensor("ident", [P, P], f32, kind="ExternalInput")
    o_d = nc.dram_tensor("out", [P, nchunk * C], f32, kind="ExternalOutput")

    # Pre-load the one ACT table set that contains ln+exp+copy, so the
    # compile-time fixpoint never needs to insert per-chunk switches.
    tset = list(get_activation_tables(nc.m.arch).keys())
    set_id = tset.index("natural_log_exp_and_others")
    nc.scalar.add_instruction(mybir.InstLoadActFuncSet(
        name=nc.get_next_instruction_name(), act_func_set_id=set_id,
        ins=[], outs=[]))

    # groups: (iv_a, [iv_c...], kind, anchor_i) — kind in {v1, tri},
    # interleaved so ScalarE-evac'd and DVE-evac'd pairs alternate.
    # Norms run separately on ScalarE (Square+accum straight off X).
    groups = []
    v1g = [(2 * i, [2 * j + 1 for j in range(C)], "v1", i) for i in range(C)]
    trig = [(2 * i, [2 * j for j in range(i + 1, C)], "tri", i)
            for i in range(C - 1)]
    for k in range(C):
        groups.append(v1g[k])
        if k < len(trig):
            groups.append(trig[k])
    # chunk-0 variant: its X arrives as two half-DMAs (crops 0-3, 4-7);
    # lead with tri sub-groups confined to crops 0-3 so DVE starts ~3us
    # before the second half lands.
    groups0 = [(0, [2, 4, 6], "tri", 0), (2, [4, 6], "tri", 1),
               (4, [6], "tri", 2)]
    for k in range(C):
        groups0.append(v1g[k])
        if k < 3:
            groups0.append((2 * k, [8, 10, 12, 14], "tri", k))
        elif k < len(trig):
            groups0.append(trig[k])

    with tile.TileContext(nc) as tc, ExitStack() as ctx:
        xp = ctx.enter_context(tc.tile_pool(name="x", bufs=2))
        pp = ctx.enter_context(tc.tile_pool(name="prod", bufs=5))
        psp = ctx.enter_context(tc.psum_pool(name="ps", bufs=7))
        scr_s = ctx.enter_context(tc.tile_pool(name="scr_s", bufs=3))
        small = ctx.enter_context(tc.tile_pool(name="small", bufs=2))
        outp = ctx.enter_context(tc.tile_pool(name="outp", bufs=1))

        loss_out = outp.tile([P, nchunk * C], f32)
        bias_t = outp.tile([P, 1], f32)
        nc.vector.memset(bias_t[:, :], half_ln10)
        ident = outp.tile([P, P], bf16)
        nc.gpsimd.dma_start(out=ident[:, :], in_=id_d.ap())  # casts f32->bf16

        x_ap = x_d.ap()

        def load_chunk(c):
            # two half-loads so chunk-0 compute can start ~6us earlier
            X = xp.tile([P, C * V, D], bf16, tag="X")
            src = x_ap[:, :, c * P:(c + 1) * P, :].rearrange(
                "i v b d -> b (i v) d")
            nc.gpsimd.dma_start(out=X[:, 0:8, :], in_=src[:, 0:8, :])
            nc.gpsimd.dma_start(out=X[:, 8:16, :], in_=src[:, 8:16, :])
            return X

        def emit_groups(X, sc_budget, glist):
            G = small.tile([P, C, C * V], f32)
            Gf = G[:, :, :].rearrange("p a b -> p (a b)")   # [128, 128] view

            # ---- norms^2 on ScalarE straight off bf16 X, then the r-chain
            nn = small.tile([P, C * V], f32)
            for iv in range(C * V):
                sq = scr_s.tile([P, D], bf16, tag="sq")
                nc.scalar.activation(out=sq[:, :], in_=X[:, iv, :],
                                     func=Act.Square,
                                     accum_out=nn[:, iv:iv + 1])
            lnn = small.tile([P, C * V], f32)
            nc.scalar.activation(out=lnn[:, :], in_=nn[:, :], func=Act.Ln)
            r = small.tile([P, C * V], f32)
            nc.scalar.activation(out=r[:, :], in_=lnn[:, :], func=Act.Exp,
                                 scale=-0.5, bias=bias_t[:, :])

            sc_quota = [0.0]    # Bresenham spread of the ScalarE evac budget
            for (iva, ivcs, kind, ai) in glist:
                n = len(ivcs)
                # ---- 1. products P[128, n, 512] bf16 (DVE TT, 2x)
                Pg = pp.tile([P, 8, D], bf16, tag="pg")
                in0 = X[:, iva, :].unsqueeze(1).broadcast_to([P, n, D])
                st = ivcs[1] - ivcs[0] if n > 1 else 1
                in1 = X[:, ivcs[0]:ivcs[-1] + 1:st, :]
                nc.vector.tensor_tensor(out=Pg[:, :n, :], in0=in0, in1=in1,
                                        op=Alu.mult)

                # ---- 2. PE: 512 -> slice_ accumulate-copy into PSUM
                PS = psp.tile([P, n, slice_], f32)
                for j in range(nsl):
                    nc.tensor.matmul(
                        out=PS[:, :, :], lhsT=ident[:, :],
                        rhs=Pg[:, :n, j * slice_:(j + 1) * slice_],
                        start=(j == 0), stop=(j == nsl - 1))

                # ---- 3. evac slice_ -> 1 into G.  The first sc_k pairs of
                # each group go to ScalarE (Copy+accum), the rest ride one
                # DVE tensor_reduce — pair-granular engine balance.
                j0, j1 = ivcs[0] // 2, ivcs[-1] // 2
                off = 1 if kind == "v1" else 0
                dst = Gf[:, 16 * ai + 2 * j0 + off:
                         16 * ai + 2 * j1 + off + 1:2]
                sc_quota[0] += n * (sc_budget / 92.0)
                sc_k = min(n, int(sc_quota[0]))
                sc_quota[0] -= sc_k
                for k in range(sc_k):
                    so = scr_s.tile([P, slice_], bf16, tag="so")
                    nc.scalar.activation(
                        out=so[:, :], in_=PS[:, k, :], func=Act.Copy,
                        accum_out=dst[:, k:k + 1])
                if sc_k < n:
                    nc.vector.tensor_reduce(
                        out=dst[:, sc_k:], in_=PS[:, sc_k:, :],
                        axis=mybir.AxisListType.X, op=Alu.add)

            return G, Gf, r

        def emit_tail(c, G, Gf, r):
            # ---- mirror v0 upper triangle -> lower: for offset k:
            #      src cols 18i+2k (i=0..8-k), dst cols 18i+16k
            for k in range(1, C):
                nk = C - k
                src_v = Gf[:, 2 * k: 2 * k + 18 * (nk - 1) + 1:18]
                dst_v = Gf[:, 16 * k: 16 * k + 18 * (nk - 1) + 1:18]
                nc.vector.tensor_copy(out=dst_v, in_=src_v)
            # ---- zero the (i,i,0) self columns (cols 18i)
            nc.vector.memset(Gf[:, 0:127:18], 0.0)

            # ---- RR[b, i, jv] = r'[b, 2i] * r'[b, jv]  (carries 1/T=10)
            RR = small.tile([P, C, C * V], f32)
            r_a = r[:, 0:C * V:2].unsqueeze(2).broadcast_to([P, C, C * V])
            r_c = r[:, :].unsqueeze(1).broadcast_to([P, C, C * V])
            nc.vector.tensor_tensor(out=RR[:, :, :], in0=r_a, in1=r_c,
                                    op=Alu.mult)

            # ---- sims = G * RR;  pos = sims[:, 18i+1]
            sims = small.tile([P, C, C * V], f32)
            nc.vector.tensor_tensor(out=sims[:, :, :], in0=G[:, :, :],
                                    in1=RR[:, :, :], op=Alu.mult)
            simsf = sims[:, :, :].rearrange("p a b -> p (a b)")
            pos = small.tile([P, C], f32)
            nc.vector.tensor_copy(out=pos[:, :], in_=simsf[:, 1:128:18])

            # ---- E = exp(sims); zero self cols; row-sum; loss = ln(D)-pos
            E = small.tile([P, C, C * V], f32)
            nc.scalar.activation(out=E[:, :, :], in_=sims[:, :, :],
                                 func=Act.Exp)
            Ef = E[:, :, :].rearrange("p a b -> p (a b)")
            nc.vector.memset(Ef[:, 0:127:18], 0.0)
            Dsum = small.tile([P, C], f32)
            nc.vector.tensor_reduce(out=Dsum[:, :], in_=E[:, :, :],
                                    axis=mybir.AxisListType.X, op=Alu.add)
            lnD = small.tile([P, C], f32)
            nc.scalar.activation(out=lnD[:, :], in_=Dsum[:, :], func=Act.Ln)
            nc.vector.tensor_tensor(
                out=loss_out[:, c * C:(c + 1) * C], in0=lnD[:, :],
                in1=pos[:, :], op=Alu.subtract)

        # software-pipeline: emit chunk c+1's product groups before chunk
        # c's (cheap but dependency-laden) tail, so the in-order DVE queue
        # never idles waiting for ScalarE's last evacuations.
        Xc = load_chunk(0)
        pending = None
        for c in range(nchunk):
            X = Xc
            if c + 1 < nchunk:
                Xc = load_chunk(c + 1)
            state = emit_groups(
                X, n_sc_last if c == nchunk - 1 else n_sc_pairs, groups)
            if pending is not None:
                emit_tail(c - 1, *pending)
            pending = state
        emit_tail(nchunk - 1, *pending)

        nc.sync.dma_start(out=o_d.ap(), in_=loss_out[:, :])

    nc.compile()
    return nc


_CACHE = {}


def _get_nc(b_loc=B_LOC, slice_=SLICE, n_sc_pairs=N_SC_PAIRS,
            n_sc_last=N_SC_LAST):
    key = (b_loc, slice_, n_sc_pairs, n_sc_last)
    if key not in _CACHE:
        _CACHE[key] = _build_kernel(b_loc, slice_, n_sc_pairs, n_sc_last)
    return _CACHE[key]


def _run(inp, trace=False):
    from concourse.bass_utils import run_bass_kernel_spmd

    nc = _get_nc()
    ident = np.eye(P, dtype=np.float32)
    in_maps = []
    for k in range(NCORES):
        shard = np.ascontiguousarray(inp[:, :, k * B_LOC:(k + 1) * B_LOC, :],
                                     dtype=np.float32)
        in_maps.append({"inp": shard, "ident": ident})
    res = run_bass_kernel_spmd(nc, in_maps, list(range(NCORES)), trace=trace)
    total = np.float64(0.0)
    for m in res.results:
        total += m["out"].astype(np.float64).sum()
    loss = np.float32(total / (C * B))
    return loss, res


def kernel(inp):
    loss, _ = _run(np.asarray(inp), trace=False)
    return loss


# revision 43
# speedup vs baseline: 1.0905x; 1.0162x over previous
"""NTXEnt (intra-sample) loss kernel for Trainium2, 8-core data-parallel.

Math (matches the jax reference):
  inp [C=8, V=2, B=4096, D=512] fp32
  xn = inp / max(||inp||_D, 1e-12)
  sim[i,b,jv] = <xn[i,0,b], xn[jv,b]> / T          (T = 0.1)
  loss[i,b]   = log( sum over jv of exp(sim) with the (j==i,v=0) self
                     column zeroed; the (i,1) pos column counts once ) - pos
  answer = mean over (i, b).

Sharding: pure data parallel over B (4096 -> 8 cores x 512).  Each core
computes per-(b,i) partial losses [128, 4*8]; the host sums in float64
and divides by C*B.

Engine split (v3).  The 92 pair-dots + 16 self-dots (norms) per 128-row
chunk are organized in anchor groups.  Per group of n pairs:
  1. DVE bf16 tensor_tensor product (2x mode): P[128, n, 512]
  2. TensorE reduces 512 -> SLICE via identity-stationary matmuls that
     accumulate D-slices into PSUM:  PS[b, k, s] = sum_j P[b, k, s + j*SLICE]
  3. final SLICE -> 1: either one DVE tensor_reduce per group (1x, but
     only n*SLICE elements) or ScalarE Copy+accum per pair, balancing
     both engines.
This beats the 1x-only scalar_tensor_tensor dot (756ns/pair measured) at
~270 (DVE) + ~220 (PE, idle otherwise) + ~100-480 (evac) ns/pair.
rsqrt via ln/exp with 1/T=10 folded in: r' = exp(-0.5*ln(nn) + 0.5*ln10).
A single ACT table set (natural_log_exp_and_others: ln/exp/copy) is
pre-loaded so walrus inserts no per-chunk table switches.
"""

import math
import os
import numpy as np

C, V, B, D = 8, 2, 4096, 512
NCORES = 8
B_LOC = B // NCORES            # 512
P = 128                        # partitions per chunk

SLICE = int(os.environ.get("NTX_SLICE", "64"))   # PE first-stage output width
N_SC_PAIRS = int(os.environ.get("NTX_SC_PAIRS", "31"))  # pairs evac'd by ScalarE
N_SC_LAST = int(os.environ.get("NTX_SC_LAST", "12"))   # ... on the last chunk


def _build_kernel(b_loc=B_LOC, slice_=SLICE, n_sc_pairs=N_SC_PAIRS,
                  n_sc_last=N_SC_LAST):
    from contextlib import ExitStack

    import concourse.bacc as bacc
    import concourse.tile as tile
    import concourse.mybir as mybir
    from concourse.hw_specs import get_activation_tables

    f32 = mybir.dt.float32
    bf16 = mybir.dt.bfloat16
    Alu = mybir.AluOpType
    Act = mybir.ActivationFunctionType

    nchunk = b_loc // P
    nsl = D // slice_
    half_ln10 = 0.5 * math.log(10.0)

    nc = bacc.Bacc("TRN2", target_bir_lowering=False, debug=False)
    x_d = nc.dram_tensor("inp", [C, V, b_loc, D], f32, kind="ExternalInput")
    id_d = nc.dram_tensor("ident", [P, P], f32, kind="ExternalInput")
    o_d = nc.dram_tensor("out", [P, nchunk * C], f32, kind="ExternalOutput")

    # Pre-load the one ACT table set that contains ln+exp+copy, so the
    # compile-time fixpoint never needs to insert per-chunk switches.
    tset = list(get_activation_tables(nc.m.arch).keys())
    set_id = tset.index("natural_log_exp_and_others")
    nc.scalar.add_instruction(mybir.InstLoadActFuncSet(
        name=nc.get_next_instruction_name(), act_func_set_id=set_id,
        ins=[], outs=[]))

    # groups: (iv_a, [iv_c...], kind, anchor_i) — kind in {v1, tri},
    # interleaved so ScalarE-evac'd and DVE-evac'd pairs alternate.
    # Norms run separately on ScalarE (Square+accum straight off X).
    groups = []
    v1g = [(2 * i, [2 * j + 1 for j in range(C)], "v1", i) for i in range(C)]
    trig = [(2 * i, [2 * j for j in range(i + 1, C)], "tri", i)
            for i in range(C - 1)]
    for k in range(C):
        groups.append(v1g[k])
        if k < len(trig):
            groups.append(trig[k])
    # chunk-0 variant: its X arrives as two half-DMAs (crops 0-3, 4-7);
    # lead with tri sub-groups confined to crops 0-3 so DVE starts ~3us
    # before the second half lands.
    groups0 = [(0, [2, 4, 6], "tri", 0), (2, [4, 6], "tri", 1),
               (4, [6], "tri", 2)]
    for k in range(C):
        groups0.append(v1g[k])
        if k < 3:
            groups0.append((2 * k, [8, 10, 12, 14], "tri", k))
        elif k < len(trig):
            groups0.append(trig[k])

    with tile.TileContext(nc) as tc, ExitStack() as ctx:
        xp = ctx.enter_context(tc.tile_pool(name="x", bufs=2))
        pp = ctx.enter_context(tc.tile_pool(name="prod", bufs=5))
        psp = ctx.enter_context(tc.psum_pool(name="ps", bufs=7))
        scr_s = ctx.enter_context(tc.tile_pool(name="scr_s", bufs=3))
        small = ctx.enter_context(tc.tile_pool(name="small", bufs=2))
        outp = ctx.enter_context(tc.tile_pool(name="outp", bufs=1))

        loss_out = outp.tile([P, nchunk * C], f32)
        bias_t = outp.tile([P, 1], f32)
        nc.vector.memset(bias_t[:, :], half_ln10)
        ident = outp.tile([P, P], bf16)
        nc.gpsimd.dma_start(out=ident[:, :], in_=id_d.ap())  # casts f32->bf16

        x_ap = x_d.ap()

        def load_chunk(c):
            # two half-loads so chunk-0 compute can start ~6us earlier
            X = xp.tile([P, C * V, D], bf16, tag="X")
            src = x_ap[:, :, c * P:(c + 1) * P, :].rearrange(
                "i v b d -> b (i v) d")
            nc.gpsimd.dma_start(out=X[:, 0:8, :], in_=src[:, 0:8, :])
            nc.gpsimd.dma_start(out=X[:, 8:16, :], in_=src[:, 8:16, :])
            return X

        def emit_groups(X, sc_budget, glist):
            G = small.tile([P, C, C * V], f32)
            Gf = G[:, :, :].rearrange("p a b -> p (a b)")   # [128, 128] view

            # ---- norms^2 on ScalarE straight off bf16 X, then the r-chain
            nn = small.tile([P, C * V], f32)
            for iv in range(C * V):
                sq = scr_s.tile([P, D], bf16, tag="sq")
                nc.scalar.activation(out=sq[:, :], in_=X[:, iv, :],
                                     func=Act.Square,
                                     accum_out=nn[:, iv:iv + 1])
            lnn = small.tile([P, C * V], f32)
            nc.scalar.activation(out=lnn[:, :], in_=nn[:, :], func=Act.Ln)
            r = small.tile([P, C * V], f32)
            nc.scalar.activation(out=r[:, :], in_=lnn[:, :], func=Act.Exp,
                                 scale=-0.5, bias=bias_t[:, :])

            sc_quota = [0.0]    # Bresenham spread of the ScalarE evac budget
            for (iva, ivcs, kind, ai) in glist:
                n = len(ivcs)
                # ---- 1. products P[128, n, 512] bf16 (DVE TT, 2x)
                Pg = pp.tile([P, 8, D], bf16, tag="pg")
                in0 = X[:, iva, :].unsqueeze(1).broadcast_to([P, n, D])
                st = ivcs[1] - ivcs[0] if n > 1 else 1
                in1 = X[:, ivcs[0]:ivcs[-1] + 1:st, :]
                nc.vector.tensor_tensor(out=Pg[:, :n, :], in0=in0, in1=in1,
                                        op=Alu.mult)

                # ---- 2. PE: 512 -> slice_ accumulate-copy into PSUM
                PS = psp.tile([P, n, slice_], f32)
                for j in range(nsl):
                    nc.tensor.matmul(
                        out=PS[:, :, :], lhsT=ident[:, :],
                        rhs=Pg[:, :n, j * slice_:(j + 1) * slice_],
                        start=(j == 0), stop=(j == nsl - 1))

                # ---- 3. evac slice_ -> 1 into G.  The first sc_k pairs of
                # each group go to ScalarE (Copy+accum), the rest ride one
                # DVE tensor_reduce — pair-granular engine balance.
                j0, j1 = ivcs[0] // 2, ivcs[-1] // 2
                off = 1 if kind == "v1" else 0
                dst = Gf[:, 16 * ai + 2 * j0 + off:
                         16 * ai + 2 * j1 + off + 1:2]
                sc_quota[0] += n * (sc_budget / 92.0)
                sc_k = min(n, int(sc_quota[0]))
                sc_quota[0] -= sc_k
                for k in range(sc_k):
                    so = scr_s.tile([P, slice_], bf16, tag="so")
                    nc.scalar.activation(
                        out=so[:, :], in_=PS[:, k, :], func=Act.Copy,
                        accum_out=dst[:, k:k + 1])
                if sc_k < n:
                    nc.vector.tensor_reduce(
                        out=dst[:, sc_k:], in_=PS[:, sc_k:, :],
                        axis=mybir.AxisListType.X, op=Alu.add)

            return G, Gf, r

        def emit_tail(c, G, Gf, r):
            # ---- mirror v0 upper triangle -> lower: for offset k:
            #      src cols 18i+2k (i=0..8-k), dst cols 18i+16k
            for k in range(1, C):
                nk = C - k
                src_v = Gf[:, 2 * k: 2 * k + 18 * (nk - 1) + 1:18]
                dst_v = Gf[:, 16 * k: 16 * k + 18 * (nk - 1) + 1:18]
                nc.vector.tensor_copy(out=dst_v, in_=src_v)
            # ---- zero the (i,i,0) self columns (cols 18i)
            nc.vector.memset(Gf[:, 0:127:18], 0.0)

            # ---- RR[b, i, jv] = r'[b, 2i] * r'[b, jv]  (carries 1/T=10)
            RR = small.tile([P, C, C * V], f32)
            r_a = r[:, 0:C * V:2].unsqueeze(2).broadcast_to([P, C, C * V])
            r_c = r[:, :].unsqueeze(1).broadcast_to([P, C, C * V])
            nc.vector.tensor_tensor(out=RR[:, :, :], in0=r_a, in1=r_c,
                                    op=Alu.mult)

            # ---- sims = G * RR;  pos = sims[:, 18i+1]
            # (flat [128,128] APs: 3D views cost ~4x in sub-stream overhead)
            sims = small.tile([P, C, C * V], f32)
            simsf = sims[:, :, :].rearrange("p a b -> p (a b)")
            RRf = RR[:, :, :].rearrange("p a b -> p (a b)")
            nc.vector.tensor_tensor(out=simsf, in0=Gf, in1=RRf, op=Alu.mult)
            pos = small.tile([P, C], f32)
            nc.vector.tensor_copy(out=pos[:, :], in_=simsf[:, 1:128:18])

            # ---- E = exp(sims); zero self cols; row-sum; loss = ln(D)-pos
            E = small.tile([P, C, C * V], f32)
            Ef = E[:, :, :].rearrange("p a b -> p (a b)")
            nc.scalar.activation(out=Ef, in_=simsf, func=Act.Exp)
            nc.vector.memset(Ef[:, 0:127:18], 0.0)
            Dsum = small.tile([P, C], f32)
            nc.vector.tensor_reduce(out=Dsum[:, :], in_=E[:, :, :],
                                    axis=mybir.AxisListType.X, op=Alu.add)
            lnD = small.tile([P, C], f32)
            nc.scalar.activation(out=lnD[:, :], in_=Dsum[:, :], func=Act.Ln)
            nc.vector.tensor_tensor(
                out=loss_out[:, c * C:(c + 1) * C], in0=lnD[:, :],
                in1=pos[:, :], op=Alu.subtract)
            if c == 1:
                # first half of the output can ship early (shrinks drain)
                nc.sync.dma_start(out=o_d.ap()[:, 0:2 * C],
                                  in_=loss_out[:, 0:2 * C])

        # software-pipeline: emit chunk c+1's product groups before chunk
        # c's (cheap but dependency-laden) tail, so the in-order DVE queue
        # never idles waiting for ScalarE's last evacuations.
        Xc = load_chunk(0)
        pending = None
        for c in range(nchunk):
            X = Xc
            if c + 1 < nchunk:
                Xc = load_chunk(c + 1)
            state = emit_groups(
                X, n_sc_last if c == nchunk - 1 else n_sc_pairs,
                groups0 if c == 0 else groups)
            if pending is not None:
                emit_tail(c - 1, *pending)
            pending = state
        emit_tail(nchunk - 1, *pending)

        nc.sync.dma_start(out=o_d.ap()[:, 2 * C:],
                          in_=loss_out[:, 2 * C:])

    nc.compile()
    return nc


_CACHE = {}


def _get_nc(b_loc=B_LOC, slice_=SLICE, n_sc_pairs=N_SC_PAIRS,
            n_sc_last=N_SC_LAST):
    key = (b_loc, slice_, n_sc_pairs, n_sc_last)
    if key not in _CACHE:
        _CACHE[key] = _build_kernel(b_loc, slice_, n_sc_pairs, n_sc_last)
    return _CACHE[key]


def _run(inp, trace=False):
    from concourse.bass_utils import run_bass_kernel_spmd

    nc = _get_nc()
    ident = np.eye(P, dtype=np.float32)
    in_maps = []
    for k in range(NCORES):
        shard = np.ascontiguousarray(inp[:, :, k * B_LOC:(k + 1) * B_LOC, :],
                                     dtype=np.float32)
        in_maps.append({"inp": shard, "ident": ident})
    res = run_bass_kernel_spmd(nc, in_maps, list(range(NCORES)), trace=trace)
    total = np.float64(0.0)
    for m in res.results:
        total += m["out"].astype(np.float64).sum()
    loss = np.float32(total / (C * B))
    return loss, res


def kernel(inp):
    loss, _ = _run(np.asarray(inp), trace=False)
    return loss
